# revision 31
# baseline (speedup 1.0000x reference)
"""Trainium2 Bass kernel for nn_LogicReasoningEncoder (GNN message passing).

Sharding: 8 cores = 4 batches x 2 target-node halves. Masked edges
(edge_mask=0) contribute nothing to the output (their exp-weight and message
gate are both zero), so the host drops them entirely (~2x less edge work).
Kept edges are bucketed into 8 fixed-capacity 128-node target blocks so every
core runs the identical SPMD program.

Host prep (index/layout/scalar-only): pre-transposed bf16 feature streams
(conf, h_r, srcz*h_r), one-hot target matrix, per-edge scalar rows computed
from weight-table gathers (attrel, gate) in chunk layout. Device does all
D-dimensional per-edge math: msg/att/den matmuls, scatter-softmax via
one-hot matmuls, layer norm.

Layer 0 needs no h gather: h0[src] = srcz * ones, so the h_src msg terms
reduce to the precomputed srcz*h_r stream and a rank-1 srcz correction.
Layers 1-2 dma_gather h from the AllGather'd node table. Scatter-softmax is
max-free (att is bounded) and normalization is deferred to a per-node divide
after aggregation; cross-core work is one pair-wise AllGather per layer.
"""

import sys
import numpy as np

for _p in ("/opt/trn_rl_repo", "/root/.axon_site/_ro/trn_rl_repo"):
    if _p not in sys.path:
        sys.path.append(_p)

import concourse.bass as bass
import concourse.mybir as mybir
from concourse import bacc, tile
from concourse.bass_utils import run_bass_kernel_spmd

F32 = mybir.dt.float32
BF16 = mybir.dt.bfloat16
I16 = mybir.dt.int16
BF16_NP = mybir.dt.np(BF16)

B, N, E, D, L, NREL, TAU = 4, 2048, 32768, 128, 3, 1000, 0.1
NHALF = N // 2          # nodes per core
NBLK = NHALF // 128     # 8 node blocks per core
P = 128


# ----------------------------------------------------------------------------
# Host-side sharding / layout prep (index manipulation + scalar math only)
# ----------------------------------------------------------------------------

def _chunkify(x, ecap):
    """[ecap] -> [128, ecap//128] with x[c*128+p] at [p, c]."""
    return np.ascontiguousarray(x.reshape(ecap // 128, 128).T)


def _wrap16(x, ecap):
    """[ecap] -> int16 [128, ecap//16]: x[i] at [i%16, i//16], tiled x8 for Q7 cores."""
    w = np.ascontiguousarray(x.astype(np.int16).reshape(ecap // 16, 16).T)
    return np.ascontiguousarray(np.tile(w, (8, 1)))


def _sigmoid(x):
    return 1.0 / (1.0 + np.exp(-x))


def prepare_core_inputs(inputs):
    ei = np.asarray(inputs["edge_index"])          # [B, 2, E] int32
    rels = np.asarray(inputs["rels"])              # [B, E]
    scores = np.asarray(inputs["scores"]).astype(np.float64)
    cm = np.asarray(inputs["edge_conf_mask"])      # [B, E] bool
    em = np.asarray(inputs["edge_mask"])           # [B, E] bool
    conf = np.asarray(inputs["conf_embeds"])       # [B, E, D] f32
    rel_table = np.asarray(inputs["rel_table"]).astype(np.float64)   # [NREL, D]
    rq = np.asarray(inputs["r_query_embed"]).astype(np.float64)      # [B, D]
    att_W = np.asarray(inputs["att_W"]).astype(np.float64)           # [L, 3D, 1]
    att_b = np.asarray(inputs["att_b"]).astype(np.float64)           # [L, 1]
    beta_W = np.asarray(inputs["beta_W"]).astype(np.float64)         # [D, 1]
    beta_b = float(np.asarray(inputs["beta_b"]).reshape(()))
    msg_W = np.asarray(inputs["msg_W"])            # [L, 5D, D]
    den_W1 = np.asarray(inputs["den_W1"])          # [3D, D]

    # --- per-core kept-edge sets: edge_mask & target-half, bucketed by block
    max_cnt = 0
    per_core = []
    for core in range(8):
        b, half = core // 2, core % 2
        base = half * NHALF
        tgt = ei[b, 1]
        sel = np.nonzero(em[b] & (tgt >= base) & (tgt < base + NHALF))[0]
        ltgt = tgt[sel] - base
        blk = ltgt >> 7
        cnts = np.bincount(blk, minlength=NBLK)
        max_cnt = max(max_cnt, int(cnts.max()))
        per_core.append((b, sel, ltgt, blk))

    e_blk = max(((max_cnt + 127) // 128) * 128, 128)
    ecap = NBLK * e_blk

    in_maps = []
    for core in range(8):
        b, sel, ltgt, blk = per_core[core]
        order = np.argsort(blk, kind="stable")
        perm = sel[order]                 # original edge ids, block-sorted
        lt = ltgt[order]
        bk = blk[order]
        cnts = np.bincount(bk, minlength=NBLK)
        slot = np.concatenate(
            [bb * e_blk + np.arange(cnts[bb]) for bb in range(NBLK)]
        ).astype(np.int64) if len(perm) else np.zeros(0, np.int64)

        n_k = len(perm)
        src_k = ei[b, 0][perm]
        rels_k = rels[b][perm]
        hr_k = rel_table[rels_k]                       # [n,D] f64
        srcz_k = (src_k == 0).astype(np.float64)

        # per-edge scalars (host): attrel rows + gate (edge_mask==1 for kept)
        a2 = att_W[:, D:2 * D, 0]                      # [L, D]
        a3 = att_W[:, 2 * D:3 * D, 0]                  # [L, D]
        attrel_k = hr_k @ a2.T + (rq[b] @ a3.T + att_b[:, 0])   # [n, L]
        beta_k = _sigmoid(hr_k @ beta_W[:, 0] + rq[b] @ beta_W[:, 0] + beta_b)
        gate_k = np.where(cm[b][perm],
                          _sigmoid((scores[b][perm] - beta_k) / TAU), 0.5)

        # scatter into padded slots
        src_p = np.zeros(ecap, np.int32)
        srcz_p = np.zeros(ecap, np.float32)
        gate_p = np.zeros(ecap, np.float32)
        # pad slots: attrel = -1e5 so exp(leaky(att)) underflows to exactly 0,
        # removing the need for an explicit edge-valid mask on device
        attrel_p = np.full((ecap, L), -1.0e5, np.float32)
        hr_p = np.zeros((ecap, D), np.float32)
        conf_p = np.zeros((ecap, D), np.float32)
        oh = np.zeros((ecap, P), np.float32)
        if n_k:
            src_p[slot] = src_k
            srcz_p[slot] = srcz_k
            gate_p[slot] = gate_k
            attrel_p[slot] = attrel_k
            hr_p[slot] = hr_k
            conf_p[slot] = conf[b][perm]
            j = lt - (slot // e_blk) * 128
            oh[slot, j] = 1.0

        # feature-major bf16 streams [P, ecap]
        hr_fm = np.ascontiguousarray(hr_p.T).astype(BF16_NP)
        hrz_fm = np.ascontiguousarray((hr_p * srcz_p[:, None]).T).astype(BF16_NP)
        conf_fm = np.ascontiguousarray(conf_p.T).astype(BF16_NP)
        oh_pm = np.ascontiguousarray(
            oh.reshape(ecap // 128, 128, 128).transpose(1, 0, 2).reshape(P, ecap)
        ).astype(BF16_NP)

        # chunk-layout scalar planes [P, C]
        attrelp = np.stack([_chunkify(attrel_p[:, k].astype(np.float32), ecap)
                            for k in range(L)], axis=1)   # [P, L, C]

        hown = np.zeros((P, NBLK * 128), np.float32)
        if core % 2 == 0:
            hown[0, 0:128] = 1.0  # node 0 lives at block 0, partition 0

        # weight-only transforms
        blk_sums = msg_W.reshape(L, 5, D, D).sum(axis=2)   # [L, 5, D] col sums
        w13 = blk_sums[:, 2, :].copy()                     # h_init_src term
        w13[0] += blk_sums[0, 1, :]                        # L0: h_src term too
        den_bias = (den_W1[D:2 * D].T @ rq[b].astype(np.float32)
                    + np.asarray(inputs["den_b1"])).reshape(D, 1)
        den_b2 = float(np.asarray(inputs["den_b2"]).reshape(()))

        m = {
            "conf_fm": conf_fm,
            "hr_fm": hr_fm,
            "hrz_fm": hrz_fm,
            "onehot_pm": oh_pm,
            "srcz_row": srcz_p.astype(BF16_NP),
            "gate_ch": _chunkify(gate_p, ecap),
            "attrelp": np.ascontiguousarray(attrelp),
            "src_wr": _wrap16(src_p, ecap),
            "hown0": hown,
            "ident_b": np.eye(P).astype(BF16_NP),
            "msg_W": msg_W.astype(np.float32),
            "msg_b_col": np.ascontiguousarray(np.asarray(inputs["msg_b"]).T).astype(np.float32),
            "w13_row": w13.reshape(1, L, D).astype(np.float32),
            "den_W0": np.ascontiguousarray(den_W1[0:D]).astype(np.float32),
            "den_WC": np.ascontiguousarray(den_W1[2 * D:3 * D]).astype(np.float32),
            "den_W2": np.asarray(inputs["den_W2"]).astype(np.float32),
            "den_bias": den_bias.astype(np.float32),
            "den_b2col": np.full((D, 1), den_b2, np.float32),
            "a1": np.ascontiguousarray(att_W[:, 0:D, 0].T).astype(np.float32),  # [D, L]
            "upd_W": np.asarray(inputs["upd_W"]).astype(np.float32),
            "upd_b_row": np.asarray(inputs["upd_b"]).reshape(L, D).astype(np.float32),
            "ln_g_row": np.asarray(inputs["ln_g"]).reshape(1, D).astype(np.float32),
            "ln_b_row": np.asarray(inputs["ln_b"]).reshape(1, D).astype(np.float32),
        }
        in_maps.append(m)
    return in_maps, ecap


# ----------------------------------------------------------------------------
# Device program
# ----------------------------------------------------------------------------

def build_program(ecap, reps=1, ablate=()):
    ab = set(ablate)
    C = ecap // 128                  # chunks
    T = ecap // 512                  # 512-edge tiles
    NSLAB = next(n for n in (6, 8, 4, 9, 3, 12, 2, 18, 1) if T % n == 0)
    SLAB = ecap // NSLAB
    ST = SLAB // 512                 # tiles per slab
    SC = SLAB // 128                 # chunks per slab
    CPB = (ecap // NBLK) // 128      # chunks per node block

    nc = bacc.Bacc("TRN2", num_devices=8, debug=False, num_swdge_queues=2)

    dp = nc.declare_dram_parameter
    conf_fm_d = dp("conf_fm", [P, ecap], BF16, isOutput=False)
    hr_fm_d = dp("hr_fm", [P, ecap], BF16, isOutput=False)
    hrz_fm_d = dp("hrz_fm", [P, ecap], BF16, isOutput=False)
    onehot_d = dp("onehot_pm", [P, ecap], BF16, isOutput=False)
    srcz_d = dp("srcz_row", [ecap], BF16, isOutput=False)
    gate_ch_d = dp("gate_ch", [P, C], F32, isOutput=False)
    attrelp_d = dp("attrelp", [P, L, C], F32, isOutput=False)
    src_wr_d = dp("src_wr", [128, ecap // 16], I16, isOutput=False)
    hown0_d = dp("hown0", [P, NBLK * 128], F32, isOutput=False)
    ident_b_d = dp("ident_b", [P, P], BF16, isOutput=False)
    msg_W_d = dp("msg_W", [L, 5 * D, D], F32, isOutput=False)
    msg_b_col_d = dp("msg_b_col", [D, L], F32, isOutput=False)
    w13_row_d = dp("w13_row", [1, L, D], F32, isOutput=False)
    den_W0_d = dp("den_W0", [D, D], F32, isOutput=False)
    den_WC_d = dp("den_WC", [D, D], F32, isOutput=False)
    den_W2_d = dp("den_W2", [D, 1], F32, isOutput=False)
    den_bias_d = dp("den_bias", [D, 1], F32, isOutput=False)
    den_b2col_d = dp("den_b2col", [D, 1], F32, isOutput=False)
    a1_d = dp("a1", [D, L], F32, isOutput=False)
    upd_W_d = dp("upd_W", [L, D, D], F32, isOutput=False)
    upd_b_row_d = dp("upd_b_row", [L, D], F32, isOutput=False)
    ln_g_row_d = dp("ln_g_row", [1, D], F32, isOutput=False)
    ln_b_row_d = dp("ln_b_row", [1, D], F32, isOutput=False)
    out_d = dp("out", [L, D], F32, isOutput=True)

    # DRAM scratch
    hhalf = nc.dram_tensor("hhalf", [NHALF, D], BF16)
    hfull = [nc.dram_tensor(f"hfull{i}", [N, D], BF16) for i in range(2)]

    AF = mybir.ActivationFunctionType
    ALU = mybir.AluOpType

    with tile.TileContext(nc) as tc:
        for _rep in range(reps):
            with (
                tc.tile_pool(name=f"res{_rep}", bufs=1) as res,
                tc.tile_pool(name=f"wgt{_rep}", bufs=1) as wgt,
            ):
                # ---------------- persistent SBUF ----------------
                hr_fm = res.tile([P, ecap], BF16)
                conf_fm = res.tile([P, ecap], BF16)
                oh_sb = res.tile([P, ecap], BF16)
                s_ch = res.tile([P, C], F32)
                gate_ch = res.tile([P, C], F32)
                attrelp = res.tile([P, L, C], F32)
                exab_ch = res.tile([P, C], BF16)
                w_ch = res.tile([P, C], BF16)
                src_wr = res.tile([128, ecap // 16], I16)
                h_tiles = [res.tile([P, NBLK, 128], F32, name=f"h_t{i}", tag=f"h_t{i}")
                           for i in range(L + 1)]

                # ---------------- weights in SBUF ----------------
                msgW = wgt.tile([P, L, 5, D], BF16)
                denW0 = wgt.tile([P, D], BF16)
                denWC = wgt.tile([P, D], BF16)
                denW2 = wgt.tile([P, 1], BF16)
                den_bias = wgt.tile([P, 1], F32)
                den_b2col = wgt.tile([P, 1], F32)
                updW = wgt.tile([P, L, D], BF16)
                a1 = wgt.tile([P, L], BF16)
                msgb = wgt.tile([P, L], F32)
                w13 = wgt.tile([1, L, D], BF16)
                updb_row = wgt.tile([1, L, D], BF16)
                ident_b = wgt.tile([P, P], BF16)
                ones_r1b = wgt.tile([1, P], BF16)
                ones_r1f = wgt.tile([1, P], F32)
                eps_col = wgt.tile([P, 1], F32)
                g_rep = wgt.tile([P, P], F32)
                b_rep = wgt.tile([P, P], F32)

                gp, sy, ve, sc, te = nc.gpsimd, nc.sync, nc.vector, nc.scalar, nc.tensor

                # ---------------- load weights / persistent planes ----------
                gp.dma_start(msgW[:], msg_W_d[:].rearrange("k (t i) o -> i k t o", i=P))
                gp.dma_start(denW0[:], den_W0_d[:])
                gp.dma_start(denWC[:], den_WC_d[:])
                gp.dma_start(denW2[:], den_W2_d[:])
                sy.dma_start(den_bias[:], den_bias_d[:])
                sy.dma_start(den_b2col[:], den_b2col_d[:])
                gp.dma_start(updW[:], upd_W_d[:].rearrange("k i o -> i k o"))
                gp.dma_start(a1[:], a1_d[:])
                sy.dma_start(msgb[:], msg_b_col_d[:])
                gp.dma_start(w13[:], w13_row_d[:])
                gp.dma_start(updb_row[:], upd_b_row_d[:].rearrange("k d -> () k d"))
                sy.dma_start(ident_b[:], ident_b_d[:])
                ve.memset(ones_r1b[:], 1.0)
                ve.memset(ones_r1f[:], 1.0)
                ve.memset(eps_col[:], 1e-5)
                sy.dma_start(gate_ch[:], gate_ch_d[:])
                sy.dma_start(attrelp[:], attrelp_d[:])
                sy.dma_start(src_wr[:], src_wr_d[:])
                sy.dma_start(h_tiles[0][:].rearrange("p b d -> p (b d)"), hown0_d[:])

                with tc.tile_pool(name=f"prep{_rep}", bufs=1, space="PSUM") as pps:
                    lng = wgt.tile([1, D], F32, name="lng_row")
                    lnb = wgt.tile([1, D], F32, name="lnb_row")
                    sy.dma_start(lng[:], ln_g_row_d[:])
                    sy.dma_start(lnb[:], ln_b_row_d[:])
                    grp = pps.tile([P, D], F32)
                    te.matmul(grp[:], ones_r1f[:], lng[:])
                    ve.tensor_copy(g_rep[:], grp[:])
                    brp = pps.tile([P, D], F32)
                    te.matmul(brp[:], ones_r1f[:], lnb[:])
                    ve.tensor_copy(b_rep[:], brp[:])

                # ---------------- layers ----------------
                for k in range(L):
                    with tc.tile_pool(name=f"ups{k}_{_rep}", bufs=1, space="PSUM") as upool:
                      # block b at a 1KiB offset: [128, 129] matmul output (cols
                      # 0-127 = weighted-msg sums, col 128 = softmax denominator)
                      # stays inside one 2KiB bank
                      usum_ps = upool.tile([P, NBLK, 256], F32)
                      with (
                        tc.tile_pool(name=f"lps{k}_{_rep}", bufs=2, space="PSUM") as lpool,
                        tc.tile_pool(name=f"trp{k}_{_rep}", bufs=1, space="PSUM") as trpool,
                        tc.tile_pool(name=f"aps{k}_{_rep}", bufs=1,
                                     space="PSUM") as apool,
                        tc.tile_pool(name=f"sl{k}_{_rep}", bufs=2) as slp,
                        tc.tile_pool(name=f"tp{k}_{_rep}", bufs=3) as tpp,
                        tc.tile_pool(name=f"rm{k}_{_rep}", bufs=ST + 3) as rmp,
                      ):
                        for s in range(NSLAB):
                            lo = s * SLAB
                            c0 = s * SC
                            if k == 0:
                                # stream in persistent planes slab-by-slab,
                                # spread across both HWDGE rings (SP + ACT)
                                sy.dma_start(hr_fm[:, lo:lo + SLAB], hr_fm_d[:, lo:lo + SLAB])
                                sc.dma_start(conf_fm[:, lo:lo + SLAB], conf_fm_d[:, lo:lo + SLAB])
                                sc.dma_start(oh_sb[:, lo:lo + SLAB], onehot_d[:, lo:lo + SLAB])
                                hrz_sl = slp.tile([P, SLAB], BF16, tag="hrz")
                                sy.dma_start(hrz_sl[:], hrz_fm_d[:, lo:lo + SLAB])
                                hsrc_sl = None
                            else:
                                hsrc_sl = slp.tile([P, SLAB], BF16, tag="hsrc", bufs=3)
                                if "gather" in ab:
                                    gp.dma_start(hsrc_sl[:], onehot_d[:, lo:lo + SLAB])
                                else:
                                    gp.dma_gather(
                                        hsrc_sl[:].rearrange("p (o e) -> p o e", o=1),
                                        hfull[k - 1][:],
                                        src_wr[:, lo // 16:(lo + SLAB) // 16],
                                        SLAB, SLAB, D, transpose=True,
                                        single_packet=False,
                                    )
                            srcz_sl = slp.tile([1, SLAB], BF16, tag="srcz")
                            sc.dma_start(srcz_sl[:], srcz_d[lo:lo + SLAB].rearrange("e -> () e"))

                            attden = apool.tile([P, 2, SC], F32, tag="attps")

                            rms = []
                            for tt in range(ST):
                                t = s * ST + tt
                                e0, f0 = t * 512, tt * 512
                                mps = lpool.tile([P, 512], F32, tag="msgps")
                                # gather-independent terms first: PE can run
                                # them before the h gather / layer boundary
                                # completes (PE executes in program order)
                                te.matmul(mps[:], msgW[:, k, 3, :], hr_fm[:, e0:e0 + 512],
                                          start=True, stop=False)
                                te.matmul(mps[:], msgW[:, k, 4, :], conf_fm[:, e0:e0 + 512],
                                          start=False, stop=False)
                                te.matmul(mps[:], w13[:, k, :], srcz_sl[:, f0:f0 + 512],
                                          start=False, stop=False)
                                if k == 0:
                                    te.matmul(mps[:], msgW[:, 0, 0, :], hrz_sl[:, f0:f0 + 512],
                                              start=False, stop=True)
                                else:
                                    prod = tpp.tile([P, 512], BF16, tag="prod")
                                    ve.tensor_tensor(prod[:], hsrc_sl[:, f0:f0 + 512],
                                                     hr_fm[:, e0:e0 + 512], ALU.mult)
                                    te.matmul(mps[:], msgW[:, k, 0, :], prod[:],
                                              start=False, stop=False)
                                    te.matmul(mps[:], msgW[:, k, 1, :], hsrc_sl[:, f0:f0 + 512],
                                              start=False, stop=True)
                                rmsg = rmp.tile([P, 512], BF16, tag="rmsg")
                                sc.activation(rmsg[:], mps[:], AF.Relu, bias=msgb[:, k:k + 1])
                                rms.append(rmsg)

                                if k == 0:
                                    dps = lpool.tile([P, 512], F32, tag="msgps",
                                                     name="dps")
                                    te.matmul(dps[:], denW0[:], hr_fm[:, e0:e0 + 512],
                                              start=True, stop=False)
                                    te.matmul(dps[:], denWC[:], conf_fm[:, e0:e0 + 512],
                                              start=False, stop=True)
                                    hid = tpp.tile([P, 512], BF16, tag="hid")
                                    sc.activation(hid[:], dps[:], AF.Relu, bias=den_bias[:])

                                for j in range(4):
                                    cc = tt * 4 + j
                                    te.matmul(attden[:, 0, cc:cc + 1],
                                              rmsg[:, 128 * j:128 * j + 128],
                                              a1[:, k:k + 1])
                                    if k == 0:
                                        te.matmul(attden[:, 1, cc:cc + 1],
                                                  hid[:, 128 * j:128 * j + 128],
                                                  denW2[:])

                            # chunk-layout scalar chain for this slab
                            if k == 0:
                                den_t = tpp.tile([P, SC], F32, tag="dent")
                                sc.activation(den_t[:], attden[:, 1, :], AF.Sigmoid,
                                              bias=den_b2col[:])
                                ve.tensor_tensor(s_ch[:, c0:c0 + SC], den_t[:],
                                                 gate_ch[:, c0:c0 + SC], ALU.mult)
                            att_t = tpp.tile([P, SC], F32, tag="attt")
                            ve.tensor_tensor(att_t[:], attden[:, 0, :],
                                             attrelp[:, k, c0:c0 + SC], ALU.add)
                            lr_t = tpp.tile([P, SC], F32, tag="lrt")
                            ve.tensor_scalar(lr_t[:], att_t[:], 0.01, None, ALU.mult)
                            ve.tensor_tensor(att_t[:], att_t[:], lr_t[:], ALU.max)
                            sc.activation(exab_ch[:, c0:c0 + SC], att_t[:], AF.Exp)
                            ve.tensor_tensor(w_ch[:, c0:c0 + SC],
                                             exab_ch[:, c0:c0 + SC],
                                             s_ch[:, c0:c0 + SC], ALU.mult)

                            # transpose + scale + scatter
                            if "scatter" not in ab:
                                wm_sl = tpp.tile([P, SC, 129], BF16, tag="wm")
                                ve.tensor_copy(
                                    wm_sl[:, :, 128:129],
                                    exab_ch[:, c0:c0 + SC].rearrange("p c -> p c ()"),
                                )
                                for tt in range(ST):
                                    t = s * ST + tt
                                    trp = trpool.tile([P, 4, P], BF16, tag="trps")
                                    for j in range(4):
                                        te.transpose(trp[:, j, :],
                                                     rms[tt][:, 128 * j:128 * j + 128],
                                                     ident_b[:])
                                    ve.tensor_tensor(
                                        wm_sl[:, 4 * tt:4 * tt + 4, 0:128], trp[:],
                                        w_ch[:, 4 * t:4 * t + 4].broadcast_to([P, 4, P]),
                                        ALU.mult,
                                    )
                                    for j in range(4):
                                        cc = 4 * t + j
                                        blk = cc // CPB
                                        ust = cc % (2 * CPB) == 0
                                        usp = cc % (2 * CPB) == 2 * CPB - 1
                                        te.matmul(usum_ps[:, blk, 0:129],
                                                  oh_sb[:, 128 * cc:128 * cc + 128],
                                                  wm_sl[:, 4 * tt + j, 0:129],
                                                  start=ust, stop=usp)

                      if "scatter" in ab:
                          for bb2 in range(NBLK):
                              te.matmul(usum_ps[:, bb2, 0:128], ident_b[:], ident_b[:],
                                        start=True, stop=True)

                      # ---------------- layer tail ----------------
                      BL = 1 if k == L - 1 else NBLK   # last layer: node-0 block only
                      with (
                          tc.tile_pool(name=f"tl{k}_{_rep}", bufs=1) as tlp,
                          tc.tile_pool(name=f"tlps{k}_{_rep}", bufs=1, space="PSUM") as tlps,
                      ):
                          sm_s = tlp.tile([P, BL], F32)
                          ve.tensor_scalar(sm_s[:], usum_ps[:, 0:BL, 128], 1e-8, None,
                                           ALU.add)
                          rsm = tlp.tile([P, BL], F32)
                          ve.reciprocal(rsm[:], sm_s[:])
                          aggr = tlp.tile([P, BL, P], BF16)
                          ve.tensor_tensor(aggr[:], usum_ps[:, 0:BL, 0:128],
                                           rsm[:].broadcast_to([P, BL, P]), ALU.mult)
                          aggrT = tlp.tile([P, BL, P], BF16)
                          trp2 = tlps.tile([P, BL, P], BF16, tag="tr2")
                          for bb in range(BL):
                              te.transpose(trp2[:, bb, :], aggr[:, bb, :], ident_b[:])
                          ve.tensor_copy(aggrT[:], trp2[:])

                          hb_ps = tlps.tile([P, BL, P], F32, tag="hb")
                          for bb in range(BL):
                              te.matmul(hb_ps[:, bb, :], aggrT[:, bb, :], updW[:, k, :],
                                        start=True, stop=False)
                              te.matmul(hb_ps[:, bb, :], ones_r1b[:], updb_row[:, k, :],
                                        start=False, stop=True)

                          hs = tlp.tile([P, BL, P], F32)
                          ve.tensor_tensor(hs[:], hb_ps[:], h_tiles[k][:, 0:BL, :], ALU.add)
                          mu = tlp.tile([P, BL], F32)
                          ve.tensor_reduce(mu[:], hs[:], mybir.AxisListType.X, ALU.add)
                          ve.tensor_scalar(mu[:], mu[:], 1.0 / P, None, ALU.mult)
                          xc = tlp.tile([P, BL, P], F32)
                          ve.tensor_tensor(xc[:], hs[:], mu[:].broadcast_to([P, BL, P]),
                                           ALU.subtract)
                          sq = tlp.tile([P, BL, P], F32)
                          sc.activation(sq[:], xc[:], AF.Square)
                          var = tlp.tile([P, BL], F32)
                          ve.tensor_reduce(var[:], sq[:], mybir.AxisListType.X, ALU.add)
                          ve.tensor_scalar(var[:], var[:], 1.0 / P, None, ALU.mult)
                          sd = tlp.tile([P, BL], F32)
                          sc.activation(sd[:], var[:], AF.Sqrt, bias=eps_col[:])
                          rsd = tlp.tile([P, BL], F32)
                          ve.reciprocal(rsd[:], sd[:])
                          hn = h_tiles[k + 1]
                          ve.tensor_tensor(hn[:, 0:BL, :], xc[:],
                                           rsd[:].broadcast_to([P, BL, P]), ALU.mult)
                          ve.tensor_tensor(hn[:, 0:BL, :], hn[:, 0:BL, :],
                                           g_rep[:].rearrange("p d -> p () d").broadcast_to([P, BL, P]),
                                           ALU.mult)
                          ve.tensor_tensor(hn[:, 0:BL, :], hn[:, 0:BL, :],
                                           b_rep[:].rearrange("p d -> p () d").broadcast_to([P, BL, P]),
                                           ALU.add)

                          sy.dma_start(out_d[k:k + 1, :], hn[0:1, 0, :])

                          if k < L - 1:
                              hstage = tlp.tile([P, NBLK, P], BF16)
                              ve.tensor_copy(hstage[:], hn[:])
                              sy.dma_start(
                                  hhalf[:].rearrange("(b p) d -> p b d", p=P),
                                  hstage[:],
                              )
                              if "cc" in ab:
                                  gp.dma_start(hfull[k % 2][0:NHALF, :], hhalf[:])
                              else:
                                  gp.collective_compute(
                                      "AllGather",
                                      ALU.bypass,
                                      replica_groups=[[0, 1], [2, 3], [4, 5], [6, 7]],
                                      ins=[hhalf[:].opt()],
                                      outs=[hfull[k % 2][:].opt()],
                                  )

    nc.compile()
    return nc


_PROGRAM_CACHE = {}


def _get_program(ecap):
    if ecap not in _PROGRAM_CACHE:
        _PROGRAM_CACHE[ecap] = build_program(ecap)
    return _PROGRAM_CACHE[ecap]


def kernel(**inputs):
    in_maps, ecap = prepare_core_inputs(inputs)
    nc = _get_program(ecap)
    res = run_bass_kernel_spmd(nc, in_maps, list(range(8)))
    outs = np.stack([np.asarray(res.results[2 * b]["out"]) for b in range(B)], axis=0)
    return outs.astype(np.float32)


# revision 54
# speedup vs baseline: 1.7581x; 1.7581x over previous
"""Trainium2 Bass kernel for nn_LogicReasoningEncoder (GNN message passing).

Sharding: 8 cores = 4 batches x 2 target-node halves. Masked edges
(edge_mask=0) contribute nothing to the output (their exp-weight and message
gate are both zero), so the host drops them entirely (~2x less edge work).
Kept edges are bucketed into 8 fixed-capacity 128-node target blocks so every
core runs the identical SPMD program.

Host prep (index/layout/scalar-only): pre-transposed bf16 feature streams
(conf, h_r, srcz*h_r), one-hot target matrix, per-edge scalar rows computed
from weight-table gathers (attrel, gate) in chunk layout. Device does all
D-dimensional per-edge math: msg/att/den matmuls, scatter-softmax via
one-hot matmuls, layer norm.

Layer 0 needs no h gather: h0[src] = srcz * ones, so the h_src msg terms
reduce to the precomputed srcz*h_r stream and a rank-1 srcz correction.
Layers 1-2 dma_gather h from the AllGather'd node table. Scatter-softmax is
max-free (att is bounded) and normalization is deferred to a per-node divide
after aggregation; cross-core work is one pair-wise AllGather per layer.
"""

import sys
import numpy as np

for _p in ("/opt/trn_rl_repo", "/root/.axon_site/_ro/trn_rl_repo"):
    if _p not in sys.path:
        sys.path.append(_p)

import concourse.bass as bass
import concourse.mybir as mybir
from concourse import bacc, tile
from concourse.bass_utils import run_bass_kernel_spmd

F32 = mybir.dt.float32
BF16 = mybir.dt.bfloat16
I16 = mybir.dt.int16
BF16_NP = mybir.dt.np(BF16)

B, N, E, D, L, NREL, TAU = 4, 2048, 32768, 128, 3, 1000, 0.1
NHALF = N // 2          # nodes per core
NBLK = NHALF // 128     # 8 node blocks per core
P = 128


# ----------------------------------------------------------------------------
# Host-side sharding / layout prep (index manipulation + scalar math only)
# ----------------------------------------------------------------------------

def _chunkify(x, ecap):
    """[ecap] -> [128, ecap//128] with x[c*128+p] at [p, c]."""
    return np.ascontiguousarray(x.reshape(ecap // 128, 128).T)


def _wrap16(x, ecap):
    """[ecap] -> int16 [128, ecap//16]: x[i] at [i%16, i//16], tiled x8 for Q7 cores."""
    w = np.ascontiguousarray(x.astype(np.int16).reshape(ecap // 16, 16).T)
    return np.ascontiguousarray(np.tile(w, (8, 1)))


def _sigmoid(x):
    return 1.0 / (1.0 + np.exp(-x))


def prepare_core_inputs(inputs):
    ei = np.asarray(inputs["edge_index"])          # [B, 2, E] int32
    rels = np.asarray(inputs["rels"])              # [B, E]
    scores = np.asarray(inputs["scores"]).astype(np.float64)
    cm = np.asarray(inputs["edge_conf_mask"])      # [B, E] bool
    em = np.asarray(inputs["edge_mask"])           # [B, E] bool
    conf = np.asarray(inputs["conf_embeds"])       # [B, E, D] f32
    rel_table = np.asarray(inputs["rel_table"]).astype(np.float64)   # [NREL, D]
    rq = np.asarray(inputs["r_query_embed"]).astype(np.float64)      # [B, D]
    att_W = np.asarray(inputs["att_W"]).astype(np.float64)           # [L, 3D, 1]
    att_b = np.asarray(inputs["att_b"]).astype(np.float64)           # [L, 1]
    beta_W = np.asarray(inputs["beta_W"]).astype(np.float64)         # [D, 1]
    beta_b = float(np.asarray(inputs["beta_b"]).reshape(()))
    msg_W = np.asarray(inputs["msg_W"])            # [L, 5D, D]
    den_W1 = np.asarray(inputs["den_W1"])          # [3D, D]

    # --- per-core kept-edge sets: edge_mask & target-half, bucketed by block
    max_cnt = 0
    per_core = []
    for core in range(8):
        b, half = core // 2, core % 2
        base = half * NHALF
        tgt = ei[b, 1]
        sel = np.nonzero(em[b] & (tgt >= base) & (tgt < base + NHALF))[0]
        ltgt = tgt[sel] - base
        blk = ltgt >> 7
        cnts = np.bincount(blk, minlength=NBLK)
        max_cnt = max(max_cnt, int(cnts.max()))
        per_core.append((b, sel, ltgt, blk))

    e_blk = max(((max_cnt + 127) // 128) * 128, 128)
    ecap = NBLK * e_blk

    in_maps = []
    for core in range(8):
        b, sel, ltgt, blk = per_core[core]
        order = np.argsort(blk, kind="stable")
        perm = sel[order]                 # original edge ids, block-sorted
        lt = ltgt[order]
        bk = blk[order]
        cnts = np.bincount(bk, minlength=NBLK)
        slot = np.concatenate(
            [bb * e_blk + np.arange(cnts[bb]) for bb in range(NBLK)]
        ).astype(np.int64) if len(perm) else np.zeros(0, np.int64)

        n_k = len(perm)
        src_k = ei[b, 0][perm]
        rels_k = rels[b][perm]
        hr_k = rel_table[rels_k]                       # [n,D] f64
        srcz_k = (src_k == 0).astype(np.float64)

        # per-edge scalars (host): attrel rows + gate (edge_mask==1 for kept)
        a2 = att_W[:, D:2 * D, 0]                      # [L, D]
        a3 = att_W[:, 2 * D:3 * D, 0]                  # [L, D]
        attrel_k = hr_k @ a2.T + (rq[b] @ a3.T + att_b[:, 0])   # [n, L]
        beta_k = _sigmoid(hr_k @ beta_W[:, 0] + rq[b] @ beta_W[:, 0] + beta_b)
        gate_k = np.where(cm[b][perm],
                          _sigmoid((scores[b][perm] - beta_k) / TAU), 0.5)

        # scatter into padded slots
        src_p = np.zeros(ecap, np.int32)
        srcz_p = np.zeros(ecap, np.float32)
        gate_p = np.zeros(ecap, np.float32)
        # pad slots: attrel = -1e5 so exp(leaky(att)) underflows to exactly 0,
        # removing the need for an explicit edge-valid mask on device
        attrel_p = np.full((ecap, L), -1.0e5, np.float32)
        hr_p = np.zeros((ecap, D), np.float32)
        conf_p = np.zeros((ecap, D), np.float32)
        oh = np.zeros((ecap, P), np.float32)
        if n_k:
            src_p[slot] = src_k
            srcz_p[slot] = srcz_k
            gate_p[slot] = gate_k
            attrel_p[slot] = attrel_k
            hr_p[slot] = hr_k
            conf_p[slot] = conf[b][perm]
            j = lt - (slot // e_blk) * 128
            oh[slot, j] = 1.0

        # feature-major bf16 streams [P, ecap]
        hr_fm = np.ascontiguousarray(hr_p.T).astype(BF16_NP)
        hrz_fm = np.ascontiguousarray((hr_p * srcz_p[:, None]).T).astype(BF16_NP)
        conf_fm = np.ascontiguousarray(conf_p.T).astype(BF16_NP)
        oh_pm = np.ascontiguousarray(
            oh.reshape(ecap // 128, 128, 128).transpose(1, 0, 2).reshape(P, ecap)
        ).astype(BF16_NP)

        # chunk-layout scalar planes [P, C]
        attrelp = np.stack([_chunkify(attrel_p[:, k].astype(np.float32), ecap)
                            for k in range(L)], axis=1)   # [P, L, C]

        hown = np.zeros((P, NBLK * 128), np.float32)
        if core % 2 == 0:
            hown[0, 0:128] = 1.0  # node 0 lives at block 0, partition 0

        # weight-only transforms
        blk_sums = msg_W.reshape(L, 5, D, D).sum(axis=2)   # [L, 5, D] col sums
        w13 = blk_sums[:, 2, :].copy()                     # h_init_src term
        w13[0] += blk_sums[0, 1, :]                        # L0: h_src term too
        den_bias = (den_W1[D:2 * D].T @ rq[b].astype(np.float32)
                    + np.asarray(inputs["den_b1"])).reshape(D, 1)
        den_b2 = float(np.asarray(inputs["den_b2"]).reshape(()))

        m = {
            "conf_fm": conf_fm,
            "hr_fm": hr_fm,
            "hrz_fm": hrz_fm,
            "onehot_pm": oh_pm,
            "srcz_row": srcz_p.astype(BF16_NP),
            "gate_ch": _chunkify(gate_p, ecap),
            "attrelp": np.ascontiguousarray(attrelp),
            "src_wr": _wrap16(src_p, ecap),
            "hown0": hown,
            "ident_b": np.eye(P).astype(BF16_NP),
            # weights pre-cast to bf16 host-side so every load is a plain
            # HWDGE DMA (no SWDGE cast) and the Pool engine stays free for
            # gather descriptor pre-emission
            "msg_W": np.ascontiguousarray(
                msg_W.reshape(L, 5, D, D).transpose(2, 0, 1, 3)).astype(BF16_NP),
            "msg_b_col": np.ascontiguousarray(np.asarray(inputs["msg_b"]).T).astype(np.float32),
            "w13_row": w13.reshape(1, L, D).astype(BF16_NP),
            "den_W0": np.ascontiguousarray(den_W1[0:D]).astype(BF16_NP),
            "den_WC": np.ascontiguousarray(den_W1[2 * D:3 * D]).astype(BF16_NP),
            "den_W2": np.asarray(inputs["den_W2"]).astype(BF16_NP),
            "den_bias": den_bias.astype(np.float32),
            "den_b2col": np.full((D, 1), den_b2, np.float32),
            "a1": np.ascontiguousarray(att_W[:, 0:D, 0].T).astype(BF16_NP),  # [D, L]
            "upd_W": np.ascontiguousarray(
                np.asarray(inputs["upd_W"]).transpose(1, 0, 2)).astype(BF16_NP),
            "upd_b_row": np.asarray(inputs["upd_b"]).reshape(1, L, D).astype(BF16_NP),
            "ln_g_row": np.asarray(inputs["ln_g"]).reshape(1, D).astype(np.float32),
            "ln_b_row": np.asarray(inputs["ln_b"]).reshape(1, D).astype(np.float32),
        }
        in_maps.append(m)
    return in_maps, ecap


# ----------------------------------------------------------------------------
# Device program
# ----------------------------------------------------------------------------

def build_program(ecap, reps=1, ablate=(), gprep="full"):
    ab = set(ablate)
    C = ecap // 128                  # chunks
    T = ecap // 512                  # 512-edge tiles
    NSLAB = next(n for n in (6, 8, 4, 9, 3, 12, 2, 18, 1) if T % n == 0)
    SLAB = ecap // NSLAB
    ST = SLAB // 512                 # tiles per slab
    SC = SLAB // 128                 # chunks per slab
    CPB = (ecap // NBLK) // 128      # chunks per node block

    nc = bacc.Bacc("TRN2", num_devices=8, debug=False, num_swdge_queues=2,
                   dynamic_dma_scratch_size=32768)

    dp = nc.declare_dram_parameter
    conf_fm_d = dp("conf_fm", [P, ecap], BF16, isOutput=False)
    hr_fm_d = dp("hr_fm", [P, ecap], BF16, isOutput=False)
    hrz_fm_d = dp("hrz_fm", [P, ecap], BF16, isOutput=False)
    onehot_d = dp("onehot_pm", [P, ecap], BF16, isOutput=False)
    srcz_d = dp("srcz_row", [ecap], BF16, isOutput=False)
    gate_ch_d = dp("gate_ch", [P, C], F32, isOutput=False)
    attrelp_d = dp("attrelp", [P, L, C], F32, isOutput=False)
    src_wr_d = dp("src_wr", [128, ecap // 16], I16, isOutput=False)
    hown0_d = dp("hown0", [P, NBLK * 128], F32, isOutput=False)
    ident_b_d = dp("ident_b", [P, P], BF16, isOutput=False)
    msg_W_d = dp("msg_W", [P, L, 5, D], BF16, isOutput=False)
    msg_b_col_d = dp("msg_b_col", [D, L], F32, isOutput=False)
    w13_row_d = dp("w13_row", [1, L, D], BF16, isOutput=False)
    den_W0_d = dp("den_W0", [D, D], BF16, isOutput=False)
    den_WC_d = dp("den_WC", [D, D], BF16, isOutput=False)
    den_W2_d = dp("den_W2", [D, 1], BF16, isOutput=False)
    den_bias_d = dp("den_bias", [D, 1], F32, isOutput=False)
    den_b2col_d = dp("den_b2col", [D, 1], F32, isOutput=False)
    a1_d = dp("a1", [D, L], BF16, isOutput=False)
    upd_W_d = dp("upd_W", [P, L, D], BF16, isOutput=False)
    upd_b_row_d = dp("upd_b_row", [1, L, D], BF16, isOutput=False)
    ln_g_row_d = dp("ln_g_row", [1, D], F32, isOutput=False)
    ln_b_row_d = dp("ln_b_row", [1, D], F32, isOutput=False)
    out_d = dp("out", [L, D], F32, isOutput=True)

    # DRAM scratch
    hhalf = nc.dram_tensor("hhalf", [NHALF, D], BF16)
    hfull = [nc.dram_tensor(f"hfull{i}", [N, D], BF16) for i in range(2)]

    AF = mybir.ActivationFunctionType
    ALU = mybir.AluOpType

    # DMA-completion sems for gather preps: bass requires one per prep, but we
    # strip it from on_update immediately (Tile's lane sem takes its place).
    # Cache per (layer, slab) so reps reuse the same 12 handles.
    _prep_sems = {}

    def _prep_sem(nc_, key):
        if key not in _prep_sems:
            _prep_sems[key] = nc_.alloc_semaphore(f"gs_{'_'.join(map(str, key))}")
        return _prep_sems[key]

    with tile.TileContext(nc) as tc:
        for _rep in range(reps):
            with (
                tc.tile_pool(name=f"res{_rep}", bufs=1) as res,
                tc.tile_pool(name=f"wgt{_rep}", bufs=1) as wgt,
            ):
                # ---------------- persistent SBUF ----------------
                hr_fm = res.tile([P, ecap], BF16)
                conf_fm = res.tile([P, ecap], BF16)
                oh_sb = res.tile([P, ecap], BF16)
                s_ch = res.tile([P, C], F32)
                gate_ch = res.tile([P, C], F32)
                attrelp = res.tile([P, L, C], F32)
                exab_ch = res.tile([P, C], BF16)
                w_ch = res.tile([P, C], BF16)
                src_wr = res.tile([128, ecap // 16], I16)
                h_tiles = [res.tile([P, NBLK, 128], F32, name=f"h_t{i}", tag=f"h_t{i}")
                           for i in range(L + 1)]
                # full-layer gather destinations (ping-pong): descriptor
                # pre-emission bakes dst addresses, so each layer's gather
                # output needs its own stable tile
                hsrc_fm = [res.tile([P, ecap], BF16, name=f"hsrc{i}", tag=f"hsrc{i}")
                           for i in range(L - 1)]

                # ---------------- weights in SBUF ----------------
                msgW = wgt.tile([P, L, 5, D], BF16)
                denW0 = wgt.tile([P, D], BF16)
                denWC = wgt.tile([P, D], BF16)
                denW2 = wgt.tile([P, 1], BF16)
                den_bias = wgt.tile([P, 1], F32)
                den_b2col = wgt.tile([P, 1], F32)
                updW = wgt.tile([P, L, D], BF16)
                a1 = wgt.tile([P, L], BF16)
                msgb = wgt.tile([P, L], F32)
                w13 = wgt.tile([1, L, D], BF16)
                updb_row = wgt.tile([1, L, D], BF16)
                ident_b = wgt.tile([P, P], BF16)
                ones_r1b = wgt.tile([1, P], BF16)
                ones_r1f = wgt.tile([1, P], F32)
                eps_col = wgt.tile([P, 1], F32)
                g_rep = wgt.tile([P, P], F32)
                b_rep = wgt.tile([P, P], F32)

                gp, sy, ve, sc, te = nc.gpsimd, nc.sync, nc.vector, nc.scalar, nc.tensor

                # ---------------- load weights / persistent planes ----------
                # all weight loads on the HWDGE rings: the Pool engine must be
                # free from t=0 so gather descriptor pre-emission runs under
                # layer-0 compute
                sy.dma_start(msgW[:], msg_W_d[:])
                sc.dma_start(denW0[:], den_W0_d[:])
                sc.dma_start(denWC[:], den_WC_d[:])
                sc.dma_start(denW2[:], den_W2_d[:])
                sy.dma_start(den_bias[:], den_bias_d[:])
                sy.dma_start(den_b2col[:], den_b2col_d[:])
                sc.dma_start(updW[:], upd_W_d[:])
                sc.dma_start(a1[:], a1_d[:])
                sy.dma_start(msgb[:], msg_b_col_d[:])
                sc.dma_start(w13[:], w13_row_d[:])
                sc.dma_start(updb_row[:], upd_b_row_d[:])
                sy.dma_start(ident_b[:], ident_b_d[:])
                ve.memset(ones_r1b[:], 1.0)
                ve.memset(ones_r1f[:], 1.0)
                ve.memset(eps_col[:], 1e-5)
                sy.dma_start(gate_ch[:], gate_ch_d[:])
                sy.dma_start(attrelp[:], attrelp_d[:])
                sy.dma_start(src_wr[:], src_wr_d[:])
                sy.dma_start(h_tiles[0][:].rearrange("p b d -> p (b d)"), hown0_d[:])

                with tc.tile_pool(name=f"prep{_rep}", bufs=1, space="PSUM") as pps:
                    lng = wgt.tile([1, D], F32, name="lng_row")
                    lnb = wgt.tile([1, D], F32, name="lnb_row")
                    sy.dma_start(lng[:], ln_g_row_d[:])
                    sy.dma_start(lnb[:], ln_b_row_d[:])
                    grp = pps.tile([P, D], F32)
                    te.matmul(grp[:], ones_r1f[:], lng[:])
                    ve.tensor_copy(g_rep[:], grp[:])
                    brp = pps.tile([P, D], F32)
                    te.matmul(brp[:], ones_r1f[:], lnb[:])
                    ve.tensor_copy(b_rep[:], brp[:])

                def emit_gather_prep(layer, s2):
                    # pre-emit the Q7 descriptor generation (the expensive
                    # part of dma_gather, ~8ns/idx) for layer `layer`'s h
                    # gather; the RAW dep on hfull defers to trigger_dma.
                    # The per-layer sem is incremented (by 16) when each
                    # slab's DMA data lands; consumers gate on it explicitly
                    # below (Tile's own lane-sem gating fires at prep time,
                    # too early, so it cannot be relied on for prepped DMAs).
                    lo2 = s2 * SLAB
                    gp.dma_gather(
                        hsrc_fm[layer - 1][:, lo2:lo2 + SLAB]
                            .rearrange("p (o e) -> p o e", o=1),
                        hfull[(layer - 1) % 2][:],
                        src_wr[:, lo2 // 16:(lo2 + SLAB) // 16],
                        SLAB, SLAB, D, transpose=True, single_packet=False,
                        prepare_only=True,
                        sem=_prep_sem(nc, (layer,)),
                    )

                if L > 1 and "gather" not in ab and gprep in ("slab", "full"):
                    if _rep == 0:
                        # the prep sems persist across NEFF executions (Tile
                        # only resets its own); clear before first use so the
                        # cumulative wait thresholds hold on every run
                        for kk in range(1, L):
                            gp.sem_clear(_prep_sem(nc, (kk,)))
                if L > 1 and "gather" not in ab and gprep == "full":
                    for s2 in range(NSLAB):
                        emit_gather_prep(1, s2)

                # ---------------- layers ----------------
                for k in range(L):
                    if k > 0 and "gather" not in ab and gprep == "full":
                        gp.trigger_dma(count=None)
                        if k < L - 1:
                            for s2 in range(NSLAB):
                                emit_gather_prep(k + 1, s2)
                    with tc.tile_pool(name=f"ups{k}_{_rep}", bufs=1, space="PSUM") as upool:
                      # block b at a 1KiB offset: [128, 129] matmul output (cols
                      # 0-127 = weighted-msg sums, col 128 = softmax denominator)
                      # stays inside one 2KiB bank
                      usum_ps = upool.tile([P, NBLK, 256], F32)
                      with (
                        tc.tile_pool(name=f"lps{k}_{_rep}", bufs=2, space="PSUM") as lpool,
                        tc.tile_pool(name=f"trp{k}_{_rep}", bufs=1, space="PSUM") as trpool,
                        tc.tile_pool(name=f"aps{k}_{_rep}", bufs=1,
                                     space="PSUM") as apool,
                        tc.tile_pool(name=f"sl{k}_{_rep}", bufs=2) as slp,
                        tc.tile_pool(name=f"tp{k}_{_rep}", bufs=3) as tpp,
                        tc.tile_pool(name=f"rm{k}_{_rep}", bufs=ST + 3) as rmp,
                      ):
                        for s in range(NSLAB):
                            lo = s * SLAB
                            c0 = s * SC
                            if k == 0:
                                # stream in persistent planes slab-by-slab,
                                # spread across both HWDGE rings (SP + ACT)
                                sy.dma_start(hr_fm[:, lo:lo + SLAB], hr_fm_d[:, lo:lo + SLAB])
                                sc.dma_start(conf_fm[:, lo:lo + SLAB], conf_fm_d[:, lo:lo + SLAB])
                                sc.dma_start(oh_sb[:, lo:lo + SLAB], onehot_d[:, lo:lo + SLAB])
                                hrz_sl = slp.tile([P, SLAB], BF16, tag="hrz")
                                sy.dma_start(hrz_sl[:], hrz_fm_d[:, lo:lo + SLAB])
                                hsrc_sl = None
                            else:
                                hsrc_sl = hsrc_fm[k - 1]
                                if "gather" in ab:
                                    gp.dma_start(hsrc_sl[:, lo:lo + SLAB],
                                                 onehot_d[:, lo:lo + SLAB])
                                elif gprep == "slab":
                                    emit_gather_prep(k, s)
                                    gp.trigger_dma(count=1)
                                elif gprep == "direct":
                                    gp.dma_gather(
                                        hsrc_sl[:, lo:lo + SLAB]
                                            .rearrange("p (o e) -> p o e", o=1),
                                        hfull[k - 1][:],
                                        src_wr[:, lo // 16:(lo + SLAB) // 16],
                                        SLAB, SLAB, D, transpose=True,
                                        single_packet=False,
                                    )
                                elif gprep == "direct2":
                                    # split each slab's gather across both
                                    # SWDGE queues to parallelize Q7 emission
                                    HS = SLAB // 2
                                    for q in range(2):
                                        gp.dma_gather(
                                            hsrc_sl[:, lo + q * HS:lo + (q + 1) * HS]
                                                .rearrange("p (o e) -> p o e", o=1),
                                            hfull[k - 1][:],
                                            src_wr[:, (lo + q * HS) // 16:
                                                      (lo + (q + 1) * HS) // 16],
                                            HS, HS, D, transpose=True,
                                            single_packet=False, queue_num=q,
                                        )
                            srcz_sl = slp.tile([1, SLAB], BF16, tag="srcz")
                            sc.dma_start(srcz_sl[:], srcz_d[lo:lo + SLAB].rearrange("e -> () e"))

                            attden = apool.tile([P, 2, SC], F32, tag="attps")

                            rms = []
                            for tt in range(ST):
                                t = s * ST + tt
                                e0, f0 = t * 512, tt * 512
                                mps = lpool.tile([P, 512], F32, tag="msgps")
                                # gather-independent terms first: PE can run
                                # them before the h gather / layer boundary
                                # completes (PE executes in program order)
                                te.matmul(mps[:], msgW[:, k, 3, :], hr_fm[:, e0:e0 + 512],
                                          start=True, stop=False)
                                te.matmul(mps[:], msgW[:, k, 4, :], conf_fm[:, e0:e0 + 512],
                                          start=False, stop=False)
                                te.matmul(mps[:], w13[:, k, :], srcz_sl[:, f0:f0 + 512],
                                          start=False, stop=False)
                                if k == 0:
                                    te.matmul(mps[:], msgW[:, 0, 0, :], hrz_sl[:, f0:f0 + 512],
                                              start=False, stop=True)
                                else:
                                    prod = tpp.tile([P, 512], BF16, tag="prod")
                                    pr = ve.tensor_tensor(prod[:], hsrc_sl[:, e0:e0 + 512],
                                                          hr_fm[:, e0:e0 + 512], ALU.mult)
                                    te.matmul(mps[:], msgW[:, k, 0, :], prod[:],
                                              start=False, stop=False)
                                    hm = te.matmul(mps[:], msgW[:, k, 1, :],
                                                   hsrc_sl[:, e0:e0 + 512],
                                                   start=False, stop=True)
                                    if tt == 0 and gprep in ("slab", "full") \
                                            and "gather" not in ab:
                                        thr = 16 * (_rep * NSLAB + s + 1)
                                        gsem = _prep_sem(nc, (k,))
                                        pr._wait_ge(gsem, thr)
                                        hm._wait_ge(gsem, thr)
                                rmsg = rmp.tile([P, 512], BF16, tag="rmsg")
                                sc.activation(rmsg[:], mps[:], AF.Relu, bias=msgb[:, k:k + 1])
                                rms.append(rmsg)

                                if k == 0:
                                    dps = lpool.tile([P, 512], F32, tag="msgps",
                                                     name="dps")
                                    te.matmul(dps[:], denW0[:], hr_fm[:, e0:e0 + 512],
                                              start=True, stop=False)
                                    te.matmul(dps[:], denWC[:], conf_fm[:, e0:e0 + 512],
                                              start=False, stop=True)
                                    hid = tpp.tile([P, 512], BF16, tag="hid")
                                    sc.activation(hid[:], dps[:], AF.Relu, bias=den_bias[:])

                                for j in range(4):
                                    cc = tt * 4 + j
                                    te.matmul(attden[:, 0, cc:cc + 1],
                                              rmsg[:, 128 * j:128 * j + 128],
                                              a1[:, k:k + 1])
                                    if k == 0:
                                        te.matmul(attden[:, 1, cc:cc + 1],
                                                  hid[:, 128 * j:128 * j + 128],
                                                  denW2[:])

                            # chunk-layout scalar chain for this slab
                            if k == 0:
                                den_t = tpp.tile([P, SC], F32, tag="dent")
                                sc.activation(den_t[:], attden[:, 1, :], AF.Sigmoid,
                                              bias=den_b2col[:])
                                ve.tensor_tensor(s_ch[:, c0:c0 + SC], den_t[:],
                                                 gate_ch[:, c0:c0 + SC], ALU.mult)
                            att_t = tpp.tile([P, SC], F32, tag="attt")
                            ve.tensor_tensor(att_t[:], attden[:, 0, :],
                                             attrelp[:, k, c0:c0 + SC], ALU.add)
                            lr_t = tpp.tile([P, SC], F32, tag="lrt")
                            ve.tensor_scalar(lr_t[:], att_t[:], 0.01, None, ALU.mult)
                            ve.tensor_tensor(att_t[:], att_t[:], lr_t[:], ALU.max)
                            sc.activation(exab_ch[:, c0:c0 + SC], att_t[:], AF.Exp)
                            ve.tensor_tensor(w_ch[:, c0:c0 + SC],
                                             exab_ch[:, c0:c0 + SC],
                                             s_ch[:, c0:c0 + SC], ALU.mult)

                            # transpose + scale + scatter
                            if "scatter" not in ab:
                                wm_sl = tpp.tile([P, SC, 129], BF16, tag="wm")
                                ve.tensor_copy(
                                    wm_sl[:, :, 128:129],
                                    exab_ch[:, c0:c0 + SC].rearrange("p c -> p c ()"),
                                )
                                for tt in range(ST):
                                    t = s * ST + tt
                                    trp = trpool.tile([P, 4, P], BF16, tag="trps")
                                    for j in range(4):
                                        te.transpose(trp[:, j, :],
                                                     rms[tt][:, 128 * j:128 * j + 128],
                                                     ident_b[:])
                                    ve.tensor_tensor(
                                        wm_sl[:, 4 * tt:4 * tt + 4, 0:128], trp[:],
                                        w_ch[:, 4 * t:4 * t + 4].broadcast_to([P, 4, P]),
                                        ALU.mult,
                                    )
                                    for j in range(4):
                                        cc = 4 * t + j
                                        blk = cc // CPB
                                        ust = cc % (2 * CPB) == 0
                                        usp = cc % (2 * CPB) == 2 * CPB - 1
                                        te.matmul(usum_ps[:, blk, 0:129],
                                                  oh_sb[:, 128 * cc:128 * cc + 128],
                                                  wm_sl[:, 4 * tt + j, 0:129],
                                                  start=ust, stop=usp)

                      if "scatter" in ab:
                          for bb2 in range(NBLK):
                              te.matmul(usum_ps[:, bb2, 0:128], ident_b[:], ident_b[:],
                                        start=True, stop=True)

                      # ---------------- layer tail ----------------
                      BL = 1 if k == L - 1 else NBLK   # last layer: node-0 block only
                      with (
                          tc.tile_pool(name=f"tl{k}_{_rep}", bufs=1) as tlp,
                          tc.tile_pool(name=f"tlps{k}_{_rep}", bufs=1, space="PSUM") as tlps,
                      ):
                          sm_s = tlp.tile([P, BL], F32)
                          ve.tensor_scalar(sm_s[:], usum_ps[:, 0:BL, 128], 1e-8, None,
                                           ALU.add)
                          rsm = tlp.tile([P, BL], F32)
                          ve.reciprocal(rsm[:], sm_s[:])
                          aggr = tlp.tile([P, BL, P], BF16)
                          ve.tensor_tensor(aggr[:], usum_ps[:, 0:BL, 0:128],
                                           rsm[:].broadcast_to([P, BL, P]), ALU.mult)
                          aggrT = tlp.tile([P, BL, P], BF16)
                          trp2 = tlps.tile([P, BL, P], BF16, tag="tr2")
                          for bb in range(BL):
                              te.transpose(trp2[:, bb, :], aggr[:, bb, :], ident_b[:])
                          ve.tensor_copy(aggrT[:], trp2[:])

                          hb_ps = tlps.tile([P, BL, P], F32, tag="hb")
                          for bb in range(BL):
                              te.matmul(hb_ps[:, bb, :], aggrT[:, bb, :], updW[:, k, :],
                                        start=True, stop=False)
                              te.matmul(hb_ps[:, bb, :], ones_r1b[:], updb_row[:, k, :],
                                        start=False, stop=True)

                          hs = tlp.tile([P, BL, P], F32)
                          ve.tensor_tensor(hs[:], hb_ps[:], h_tiles[k][:, 0:BL, :], ALU.add)
                          mu = tlp.tile([P, BL], F32)
                          ve.tensor_reduce(mu[:], hs[:], mybir.AxisListType.X, ALU.add)
                          ve.tensor_scalar(mu[:], mu[:], 1.0 / P, None, ALU.mult)
                          xc = tlp.tile([P, BL, P], F32)
                          ve.tensor_tensor(xc[:], hs[:], mu[:].broadcast_to([P, BL, P]),
                                           ALU.subtract)
                          sq = tlp.tile([P, BL, P], F32)
                          sc.activation(sq[:], xc[:], AF.Square)
                          var = tlp.tile([P, BL], F32)
                          ve.tensor_reduce(var[:], sq[:], mybir.AxisListType.X, ALU.add)
                          ve.tensor_scalar(var[:], var[:], 1.0 / P, None, ALU.mult)
                          sd = tlp.tile([P, BL], F32)
                          sc.activation(sd[:], var[:], AF.Sqrt, bias=eps_col[:])
                          rsd = tlp.tile([P, BL], F32)
                          ve.reciprocal(rsd[:], sd[:])
                          hn = h_tiles[k + 1]
                          ve.tensor_tensor(hn[:, 0:BL, :], xc[:],
                                           rsd[:].broadcast_to([P, BL, P]), ALU.mult)
                          ve.tensor_tensor(hn[:, 0:BL, :], hn[:, 0:BL, :],
                                           g_rep[:].rearrange("p d -> p () d").broadcast_to([P, BL, P]),
                                           ALU.mult)
                          ve.tensor_tensor(hn[:, 0:BL, :], hn[:, 0:BL, :],
                                           b_rep[:].rearrange("p d -> p () d").broadcast_to([P, BL, P]),
                                           ALU.add)

                          sy.dma_start(out_d[k:k + 1, :], hn[0:1, 0, :])

                          if k < L - 1:
                              hstage = tlp.tile([P, NBLK, P], BF16)
                              ve.tensor_copy(hstage[:], hn[:])
                              sy.dma_start(
                                  hhalf[:].rearrange("(b p) d -> p b d", p=P),
                                  hstage[:],
                              )
                              if "cc" in ab:
                                  gp.dma_start(hfull[k % 2][0:NHALF, :], hhalf[:])
                              else:
                                  gp.collective_compute(
                                      "AllGather",
                                      ALU.bypass,
                                      replica_groups=[[0, 1], [2, 3], [4, 5], [6, 7]],
                                      ins=[hhalf[:].opt()],
                                      outs=[hfull[k % 2][:].opt()],
                                  )

    nc.compile()
    return nc


_PROGRAM_CACHE = {}


def _get_program(ecap):
    if ecap not in _PROGRAM_CACHE:
        _PROGRAM_CACHE[ecap] = build_program(ecap)
    return _PROGRAM_CACHE[ecap]


def kernel(**inputs):
    in_maps, ecap = prepare_core_inputs(inputs)
    nc = _get_program(ecap)
    res = run_bass_kernel_spmd(nc, in_maps, list(range(8)))
    outs = np.stack([np.asarray(res.results[2 * b]["out"]) for b in range(B)], axis=0)
    return outs.astype(np.float32)
